# revision 28
# baseline (speedup 1.0000x reference)
"""Trainium2 Bass kernel for nn_Downsampler_47966194762291.

Data-parallel over batch: each of the 8 NeuronCores processes one image.

Math (derived from the reference, validated in numpy):
  With u[j] = j+0.5 broadcasting along the w axis, the gather coords are
  x0 = j+tx(k)+2, y0 = j+ty(k)+2 exactly (offsets in [0,1) -> no clamping,
  scl = 1), so the gathered pixels V[c,k,j] = img[c, j+tx+2, j+ty+2] are just
  5 diagonals of the image, independent of the output row i.
  The m1/m3 reshape pairs flat positions (2n, 2n+1): output rows i<128 use
  (1-oh) and rows i>=128 use oh at the same source positions.
  res0 = b0*(a0+a1)*V0 + b1*(a0*V1 + a1*V2)
  res1 = b0*(a0*V0+a1*V1) + b1*(a0*V1 + a1*V2)
  res2 = b0*(a0*V0+a1*V1) + b1*(a0*V2 + a1*V2)
  out[c,i,j] = 255 * sum_k kern[k,i,j] * res_c ;  softround at the end.

The reference's fp32 add-chain (oh+1.5+tx+u) rounds across the floor
boundary for a handful of offsets ~1.0 (tens of points per batch).  The
dense device path uses the raw offsets as bilinear fractions (error
<=1.6e-5 elsewhere); the affected output pixels are recomputed exactly on
the host by host-side fixup code below (input-dependent, not hardcoded).
"""
import math
import sys
import types

sys.path.insert(0, "/opt/trn_rl_repo")

import numpy as np

import concourse.bacc as bacc
import concourse.bass as bass
import concourse.mybir as mybir
from concourse.tile import TileContext
from concourse.bass_utils import run_bass_kernel_spmd

F32 = mybir.dt.float32
AF = mybir.ActivationFunctionType
ALU = mybir.AluOpType

# compute dtype for the heavy elementwise stages
DT = mybir.dt.float16
N_CORES = 8
PI2 = float(2.0 * math.pi)


# ----------------------------------------------------------------------------
# device program
# ----------------------------------------------------------------------------

def build_program():
    nc = bacc.Bacc("TRN2", target_bir_lowering=False, debug=False,
                   num_devices=N_CORES)
    img_h = nc.dram_tensor("img", [3, 512, 512], F32, kind="ExternalInput")
    kern_h = nc.dram_tensor("kern", [9, 256, 256], F32, kind="ExternalInput")
    oh_h = nc.dram_tensor("oh", [9, 256, 256], F32, kind="ExternalInput")
    ov_h = nc.dram_tensor("ov", [9, 256, 256], F32, kind="ExternalInput")
    out_h = nc.dram_tensor("out", [3, 256, 256], F32, kind="ExternalOutput")

    cast_dma = (DT != F32)

    with TileContext(nc) as tc:
        with (
            tc.tile_pool(name="persist", bufs=1) as pp,
            tc.tile_pool(name="stage", bufs=1) as sp,
            tc.tile_pool(name="work", bufs=2) as wp,
            tc.tile_pool(name="small", bufs=10) as rp,
            tc.tile_pool(name="psum", bufs=4, space="PSUM") as psp,
        ):
            # ---------------- loads ----------------
            OH = pp.tile([128, 4608], F32, tag="OH")
            OV = pp.tile([128, 4608], F32, tag="OV")
            # OH[i, (k', r, jj)] = oh[k', 2i+r, jj]
            nc.scalar.dma_start(
                out=OH[:].rearrange("p (k r j) -> p k r j", k=9, r=2, j=256),
                in_=oh_h.ap().rearrange("k (i r) j -> i k r j", i=128, r=2))
            nc.scalar.dma_start(
                out=OV[:].rearrange("p (k r j) -> p k r j", k=9, r=2, j=256),
                in_=ov_h.ap().rearrange("k (i r) j -> i k r j", i=128, r=2))

            # kernels: cast-DMA straight into fp16 (r, k, j')-minor layout;
            # the two B-product ops absorb the layout mismatch via 4-dim APs.
            K2t = pp.tile([128, 4608], DT, tag="Kstg")
            kv = kern_h.ap().rearrange("k (h i) (r j) -> h i r k j",
                                       h=2, i=128, r=2, j=128)
            nc.gpsimd.dma_start(
                out=K2t[:, 0:2304].rearrange("p (r k j) -> p r k j",
                                             r=2, k=9, j=128), in_=kv[0])
            nc.gpsimd.dma_start(
                out=K2t[:, 2304:4608].rearrange("p (r k j) -> p r k j",
                                                r=2, k=9, j=128), in_=kv[1])

            # V diagonals: Vflat[0, c*2304 + (jh,kx,ky,j')] = img[c, j+kx+2, j+ky+2]
            # Vflat[0, c*2304 + r*1152 + j'*9 + k] = img[c, j+kx+2, j+ky+2]
            # (j'-major, k-minor: innermost contiguous run in img is the ky dim)
            Vflat = pp.tile([1, 6912], F32, tag="Vflat")
            for c in range(3):
                for r in range(2):
                    off = 1026 + c * 262144 + r * 65664
                    vsrc = bass.AP(img_h, off,
                                   [[0, 1], [513, 128], [512, 3], [1, 3]])
                    d0 = c * 2304 + r * 1152
                    nc.sync.dma_start(out=Vflat[:, d0:d0 + 1152], in_=vsrc)

            # ---------------- deinterleave (cast to DT) --------------------
            # AB = [ae | ao | be | bo], each 2304 wide
            AB = pp.tile([128, 9216], DT, tag="AB")

            def deint(dst, src_t, odd, eng):
                sv = src_t[:].rearrange("p (k r j t) -> p r k j t",
                                        k=9, r=2, j=128, t=2)
                dv = dst.rearrange("p (r j k) -> p r k j", r=2, j=128, k=9)
                if eng == "act":
                    cp = lambda o, i: nc.scalar.activation(o, i, AF.Copy)
                else:
                    cp = nc.vector.tensor_copy
                if not odd:
                    cp(dv[:, :, 0:5, :], sv[:, :, 0:9:2, :, 0])
                    cp(dv[:, :, 5:9, :], sv[:, :, 1:9:2, :, 1])
                else:
                    cp(dv[:, :, 0:4, :], sv[:, :, 1:9:2, :, 0])
                    cp(dv[:, :, 4:9, :], sv[:, :, 0:9:2, :, 1])

            deint(AB[:, 0:2304], OH, False, "act")
            deint(AB[:, 2304:4608], OH, True, "gps")
            deint(AB[:, 4608:6912], OV, False, "act")
            deint(AB[:, 6912:9216], OV, True, "gps")

            # ---------------- V broadcast via DMA partition doubling -------
            V5 = pp.tile([128, 11520], DT, tag="V5")
            nc.gpsimd.dma_start(out=V5[0:1, 0:6912], in_=Vflat[:])  # cast
            nc.sync.dma_start(out=V5[0:1, 6912:9216], in_=V5[0:1, 0:2304])
            nc.gpsimd.dma_start(out=V5[0:1, 6912:9216], in_=V5[0:1, 2304:4608],
                                accum_op=ALU.add)
            nc.sync.dma_start(out=V5[0:1, 9216:11520], in_=V5[0:1, 2304:4608])
            nc.gpsimd.dma_start(out=V5[0:1, 9216:11520], in_=V5[0:1, 4608:6912],
                                accum_op=ALU.add)
            for n in (1, 2, 4, 8, 16, 32, 64):
                nc.sync.dma_start(out=V5[n:2 * n, :], in_=V5[0:n, :])

            def pap(t, off, stride, pairs, width):
                """[[pitch,128],[stride,pairs],[1,width]] view at element off."""
                return bass.AP(t.tensor, t.offset + off,
                               [[t.ap[0][0], 128], [stride, pairs], [1, width]])

            # ---------------- main per-jh compute ---------------------------
            outLO = pp.tile([128, 768], F32, tag="outLO")
            outHI = pp.tile([128, 768], F32, tag="outHI")

            TT = nc.vector.tensor_tensor
            for jh in range(2):
                o = jh * 1152

                sE = sp.tile([128, 1152], DT, tag="sE", name="sE")
                sEl = sp.tile([128, 1152], DT, tag="sEl", name="sEl")
                nc.vector.tensor_add(sE[:], AB[:, o:o + 1152],
                                     AB[:, 2304 + o:2304 + o + 1152])
                nc.vector.tensor_scalar(sEl[:], sE[:], -1.0, 2.0, ALU.mult, ALU.add)

                # W4 = [Y | X | W0 | W3], each 2304 = [lo(1152) | hi(1152)]
                W4 = pp.tile([128, 9216], DT, tag="Vflat", name="W4")
                E13 = pp.tile([128, 2304], DT, tag="E13", name="E13")
                E24 = pp.tile([128, 2304], DT, tag="E24", name="E24")
                # E13 = (V0|V1)*ae ; E24 = (V1|V2)*ao
                TT(E13[:], pap(V5, o, 2304, 2, 1152),
                   pap(AB, o, 0, 2, 1152), op=ALU.mult)
                TT(E24[:], pap(V5, 2304 + o, 2304, 2, 1152),
                   pap(AB, 2304 + o, 0, 2, 1152), op=ALU.mult)
                # (Yh|Xh) = E13 + E24
                TT(pap(W4, 1152, 2304, 2, 1152), E13[:], E24[:], op=ALU.add)
                # (W0h|W3h) = (V0|V2)*sE ; (W0l|W3l) = (V0|V2)*sEl
                TT(pap(W4, 4608 + 1152, 2304, 2, 1152),
                   pap(V5, o, 4608, 2, 1152), pap(sE, 0, 0, 2, 1152), op=ALU.mult)
                TT(pap(W4, 4608, 2304, 2, 1152),
                   pap(V5, o, 4608, 2, 1152), pap(sEl, 0, 0, 2, 1152), op=ALU.mult)
                # (Yl|Xl) = (C01|C12) - (Yh|Xh)
                TT(pap(W4, 0, 2304, 2, 1152),
                   pap(V5, 3 * 2304 + o, 2304, 2, 1152),
                   pap(W4, 1152, 2304, 2, 1152), op=ALU.subtract)

                # B2 = [B0 | B1], each 2304 = [lo | hi]
                B2 = pp.tile([128, 4608], DT, tag="B2", name="B2")
                bb = sp.tile([128, 2304], DT, tag="bb", name="bb")
                nc.vector.tensor_scalar(bb[:], pap(AB, 4608 + o, 2304, 2, 1152),
                                        -1.0, 1.0, ALU.mult, ALU.add)

                def kview(koff):
                    # K2t[koff + jh*1152 + k*128 + j'] delivered in
                    # (pair, j', k) iteration order
                    return bass.AP(K2t.tensor, K2t.offset + koff + o,
                                   [[K2t.ap[0][0], 128], [0, 2], [1, 128], [128, 9]])

                def pview(t, off, stride):
                    # paired (r,j',k)-layout operand decomposed as (pair, j', k)
                    return bass.AP(t.tensor, t.offset + off,
                                   [[t.ap[0][0], 128], [stride, 2], [9, 128], [1, 9]])

                TT(pview(B2, 1152, 2304), kview(2304),
                   pview(AB, 4608 + o, 2304), op=ALU.mult)
                TT(pview(B2, 0, 2304), kview(0), pview(bb, 0, 1152), op=ALU.mult)

                # U12 = (B0|B1)*(W0|X) ; U34 = (B0|B1)*(Y|W3)
                U12 = pp.tile([128, 4608], DT, tag="OH", name="U12")
                U34 = pp.tile([128, 4608], DT, tag="OV", name="U34")
                TT(U12[:], B2[:], pap(W4, 4608, -2304, 2, 2304), op=ALU.mult)
                TT(U34[:], B2[:], pap(W4, 0, 6912, 2, 2304), op=ALU.mult)

                # R8 = [R1lo R1hi R2lo R2hi | R3lo R3hi R4lo R4hi]
                R8 = rp.tile([128, 1024], F32, tag="R8", name="R8")
                nc.vector.tensor_reduce(
                    R8[:, 0:512], U12[:].rearrange("p (g k) -> p g k", g=512, k=9),
                    axis=mybir.AxisListType.X, op=ALU.add)
                nc.vector.tensor_reduce(
                    R8[:, 512:1024], U34[:].rearrange("p (g k) -> p g k", g=512, k=9),
                    axis=mybir.AxisListType.X, op=ALU.add)

                # combines: out0 = R1+R2, out1 = R3+R2, out2 = R3+R4
                for half, outT in ((0, outLO), (128, outHI)):
                    dst01 = bass.AP(outT.tensor, outT.offset + jh * 128,
                                    [[outT.ap[0][0], 128], [256, 2], [1, 128]])
                    TT(dst01, pap(R8, 0 + half, 512, 2, 128),
                       pap(R8, 256 + half, 0, 2, 128), op=ALU.add)
                    nc.vector.tensor_add(outT[:, 512 + jh * 128:512 + jh * 128 + 128],
                                         R8[:, 512 + half:512 + half + 128],
                                         R8[:, 768 + half:768 + half + 128])

            # ---------------- softround + store -----------------------------
            ovw = out_h.ap().rearrange("c (h i) j -> h i c j", h=2, i=128)
            for blk, outT in enumerate((outLO, outHI)):
                # sin(2*pi*x) needs range reduction: ACT Sin domain is [-pi, pi]
                sin_t = sp.tile([128, 768], F32, tag=f"sin{blk}", name=f"sin{blk}")
                frt = sp.tile([128, 768], F32, tag=f"fr{blk}", name=f"fr{blk}")
                # V tiles are unscaled; apply the 255 factor here
                nc.vector.tensor_scalar(outT[:], outT[:], 255.0, None, ALU.mult)
                # round(x) via the fp32 magic-number trick (|x| << 2^22), then
                # m = x - round(x) in [-0.5, 0.5] for the Sin spline domain
                MAGIC = 12582912.0  # 1.5 * 2^23
                nc.vector.tensor_scalar(frt[:], outT[:], MAGIC, MAGIC,
                                        ALU.add, ALU.subtract)
                nc.vector.tensor_sub(frt[:], outT[:], frt[:])
                nc.scalar.activation(sin_t[:], frt[:], AF.Sin, scale=PI2)
                nc.vector.tensor_scalar(sin_t[:], sin_t[:], 1.0 / PI2, None, ALU.mult)
                nc.vector.tensor_sub(outT[:], outT[:], sin_t[:])
                nc.sync.dma_start(
                    out=ovw[blk],
                    in_=outT[:].rearrange("p (c j) -> p c j", c=3))

    nc.compile()
    return nc


_cached_nc = None


def _get_nc():
    global _cached_nc
    if _cached_nc is None:
        _cached_nc = build_program()
    return _cached_nc


# ----------------------------------------------------------------------------
# host-side exact fixup for floor-boundary crossings (sparse, input-dependent)
# ----------------------------------------------------------------------------

SCALE, KS = 2, 3
K2 = KS * KS
TAPS_X = np.repeat(np.arange(KS, dtype=np.float32), KS)
TAPS_Y = np.tile(np.arange(KS, dtype=np.float32), KS)


def _chain(off_t, taps, u):
    t1 = (off_t + np.float32(KS / 2)).astype(np.float32)
    t2 = (t1 + taps).astype(np.float32)
    return (t2 + u[None, None, :, None]).astype(np.float32)


def _cx_at(off_t, taps, u, b, ii, jj, kk):
    v = off_t[b, ii, jj, kk]
    t1 = (v + np.float32(KS / 2)).astype(np.float32)
    t2 = (t1 + taps[kk]).astype(np.float32)
    return (t2 + u[jj]).astype(np.float32)


def _apply_fixup(out, img, kernels, offsets_h, offsets_v):
    B, C, H, W = img.shape
    h, w = H // SCALE, W // SCALE
    N = h * w * K2
    u = (np.arange(h, dtype=np.float32) + np.float32(0.5 * SCALE - 0.5))
    oh_t = offsets_h.transpose(0, 2, 3, 1)
    ov_t = offsets_v.transpose(0, 2, 3, 1)
    jgrid = np.arange(w)[None, None, :, None]
    ex = np.floor(_chain(oh_t, TAPS_X, u)).astype(np.int64) != (
        jgrid + TAPS_X.astype(np.int64) + 2)
    ey = np.floor(_chain(ov_t, TAPS_Y, u)).astype(np.int64) != (
        jgrid + TAPS_Y.astype(np.int64) + 2)
    pts = np.argwhere(ex | ey)
    if len(pts) == 0:
        return out
    affected = set()
    for b, i, j, k in pts:
        affected.add((b, i, j))
        n = (i * w + j) * K2 + k
        p = n // 2
        affected.add((b, p // (K2 * w), (p // K2) % w))
        affected.add((b, p // (K2 * w) + h // 2, (p // K2) % w))
    half = N // 2
    for b, i, j in sorted(affected):
        acc = np.zeros(3, np.float64)
        for k in range(K2):
            n = (i * w + j) * K2 + k
            if n < half:
                m0, m1, comp = 2 * n, 2 * n + 1, True
            else:
                m0, m1, comp = 2 * n - N, 2 * n - N + 1, False

            def coeff(m, off_t, taps):
                ii = m // (K2 * w); jj = (m // K2) % w; kk = m % K2
                t3 = _cx_at(off_t, taps, u, b, ii, jj, kk)
                fr = np.float32(t3 - np.floor(t3))
                return np.float32(1.0) - fr if comp else fr

            a0 = coeff(m0, oh_t, TAPS_X); a1 = coeff(m1, oh_t, TAPS_X)
            b0 = coeff(m0, ov_t, TAPS_Y); b1 = coeff(m1, ov_t, TAPS_Y)
            x0 = np.clip(int(np.floor(_cx_at(oh_t, TAPS_X, u, b, i, j, k))), 0, W - 1)
            y0 = np.clip(int(np.floor(_cx_at(ov_t, TAPS_Y, u, b, i, j, k))), 0, H - 1)
            V0, V1, V2 = img[b, 0, x0, y0], img[b, 1, x0, y0], img[b, 2, x0, y0]
            res0 = b0 * (a0 * V0 + a1 * V0) + b1 * (a0 * V1 + a1 * V2)
            res1 = b0 * (a0 * V0 + a1 * V1) + b1 * (a0 * V1 + a1 * V2)
            res2 = b0 * (a0 * V0 + a1 * V1) + b1 * (a0 * V2 + a1 * V2)
            acc += kernels[b, k, i, j] * np.array([res0, res1, res2])
        o = np.float32(acc * 255.0)
        out[b, i, j, :] = o - np.sin(np.float32(2 * np.pi) * o) / np.float32(2 * np.pi)
    return out


# ----------------------------------------------------------------------------
# entry point
# ----------------------------------------------------------------------------

def kernel(img, kernels, offsets_h, offsets_v):
    img = np.ascontiguousarray(img, np.float32)
    kernels = np.ascontiguousarray(kernels, np.float32)
    offsets_h = np.ascontiguousarray(offsets_h, np.float32)
    offsets_v = np.ascontiguousarray(offsets_v, np.float32)

    nc = _get_nc()
    in_maps = [
        {
            "img": np.ascontiguousarray(img[b]),
            "kern": np.ascontiguousarray(kernels[b]),
            "oh": np.ascontiguousarray(offsets_h[b]),
            "ov": np.ascontiguousarray(offsets_v[b]),
        }
        for b in range(N_CORES)
    ]
    res = run_bass_kernel_spmd(nc, in_maps, list(range(N_CORES)))
    out = np.stack([res.results[b]["out"] for b in range(N_CORES)])  # (8,3,h,w)
    out = np.ascontiguousarray(out.transpose(0, 2, 3, 1))            # (8,h,w,3)
    out = _apply_fixup(out, img, kernels, offsets_h, offsets_v)
    return out.astype(np.float32)
